# revision 33
# baseline (speedup 1.0000x reference)
"""GCN encoder (3-layer) on 8 Trainium2 NeuronCores.

Instruction-count-minimized design (this stack costs ~0.1ms per engine
instruction, so everything is batched):
- Nodes permuted (degree-sorted, snake-dealt) across 8 cores; each core owns
  6272 table rows (6250 real + 22 zero pad rows used as gather-pad targets).
- Layer tables (bf16, row-major [50176, 128]) hold dinv[s]*x[s] for layer 1
  and dinv[s]*(h @ W_next)[s] for later layers; rebuilt per shard and
  AllGathered (3 collectives per iteration including the input one).
- Chunks of dst blocks use a uniform ELL width per chunk (KA/KB), so each
  chunk needs only: 2 dma_gathers + 2 batched DVE reductions + a short batched
  DVE epilogue.
- Layers 1-2 gather in transpose mode (features on partitions): the reduction
  axis is contiguous and the [feat, dst] orientation feeds W-matmuls with dst
  as the 512-wide moving dimension (1 matmul per 512 nodes for h = acc@W, one
  per 128-node block for the table rebuild, no PE transposes anywhere).
- Layer 3 gathers in normal mode ([dst, feat] on partitions) so the output
  lands row-major; its reduction is strided.
- int16 gather indices can't span 50176 rows, so each gather is split into a
  lo call (rows of cores 0-4) and a hi call (cores 3-7); flexible sources on
  cores 3-4 balance the two.
"""
import os

import numpy as np
import ml_dtypes

N = 50000
D = 128
NCORES = 8
BPC = 49                    # blocks per core
NLOC = BPC * 128            # table rows per core (6272)
NREAL = 6250                # real nodes per core
NTAB = NCORES * NLOC        # 50176
LO_SIZE = 5 * NLOC          # lo gather region: cores 0-4 (31360 <= 32768)
HI_BASE = 3 * NLOC          # hi gather region: cores 3-7 (rows 18816..50176)
S_MAX = 224                 # max gathered slots resident per chunk

BF16 = ml_dtypes.bfloat16


class Prep:
    pass


def preprocess(x: np.ndarray, edge_index: np.ndarray) -> Prep:
    pr = Prep()
    src = np.asarray(edge_index[0], dtype=np.int64)
    dst = np.asarray(edge_index[1], dtype=np.int64)
    all_src = np.concatenate([src, np.arange(N, dtype=np.int64)])
    all_dst = np.concatenate([dst, np.arange(N, dtype=np.int64)])

    deg = np.bincount(all_dst, minlength=N).astype(np.int64)  # >= 1 (self loop)
    dinv = (1.0 / np.sqrt(deg.astype(np.float64))).astype(np.float32)

    # snake-deal nodes (by degree desc) to the 8 cores
    order = np.argsort(-deg, kind="stable")
    snake = np.concatenate([np.arange(NCORES), np.arange(NCORES - 1, -1, -1)])
    cores_seq = np.tile(snake, (N + 2 * NCORES - 1) // (2 * NCORES))[:N]
    core_of = np.empty(N, dtype=np.int64)
    core_of[order] = cores_seq

    n_lo3 = np.bincount(all_dst, weights=(core_of[all_src] < 3).astype(np.float64),
                        minlength=N).astype(np.int64)
    n_flex = np.bincount(all_dst,
                         weights=((core_of[all_src] >= 3) & (core_of[all_src] < 5)).astype(np.float64),
                         minlength=N).astype(np.int64)
    a_bal = n_lo3 + np.clip(deg // 2 - n_lo3, 0, n_flex)

    tpos = np.empty(N, dtype=np.int64)
    node_of_pos = np.full(NTAB, -1, dtype=np.int64)
    for c in range(NCORES):
        nodes = np.where(core_of == c)[0]
        o = np.lexsort((-a_bal[nodes], -deg[nodes]))
        ranked = nodes[o]
        assert len(ranked) == NREAL
        tpos[ranked] = c * NLOC + np.arange(NREAL)
        node_of_pos[c * NLOC:c * NLOC + NREAL] = ranked

    eorder = np.argsort(all_dst, kind="stable")
    src_tpos_sorted = tpos[all_src[eorder]]
    counts = np.bincount(all_dst, minlength=N)
    offs = np.zeros(N + 1, dtype=np.int64)
    offs[1:] = np.cumsum(counts)

    # per-block minimax lo/hi split (coordinated across cores; shared NEFF)
    A_min = np.zeros(BPC, dtype=np.int64)
    B_min = np.zeros(BPC, dtype=np.int64)
    D_max = np.zeros(BPC, dtype=np.int64)
    for b in range(BPC):
        pos = (np.arange(NCORES)[:, None] * NLOC + b * 128 + np.arange(128)[None, :]).ravel()
        nn = node_of_pos[pos]
        nn = nn[nn >= 0]
        A_min[b] = n_lo3[nn].max()
        B_min[b] = (deg[nn] - n_lo3[nn] - n_flex[nn]).max()
        D_max[b] = deg[nn].max()
    C_star = np.maximum(D_max, A_min + B_min)
    B_star = np.maximum(B_min, C_star - A_min)

    lo_lists = [[[None] * 128 for _ in range(BPC)] for _ in range(NCORES)]
    hi_lists = [[[None] * 128 for _ in range(BPC)] for _ in range(NCORES)]
    Ka = np.zeros(BPC, dtype=np.int64)
    Kb = np.zeros(BPC, dtype=np.int64)
    empty = np.empty(0, dtype=np.int64)
    for c in range(NCORES):
        for b in range(BPC):
            bstar = int(B_star[b])
            for p in range(128):
                pos = c * NLOC + b * 128 + p
                n = node_of_pos[pos]
                if n < 0:
                    lo_lists[c][b][p] = empty
                    hi_lists[c][b][p] = empty
                    continue
                s = src_tpos_sorted[offs[n]:offs[n + 1]]
                is_flex = (s >= HI_BASE) & (s < LO_SIZE)
                flex = s[is_flex]
                a_p = max(int(n_lo3[n]), int(deg[n]) - bstar)
                nflex_lo = a_p - int(n_lo3[n])
                lo = np.concatenate([s[s < HI_BASE], flex[:nflex_lo]])
                hi = np.concatenate([flex[nflex_lo:], s[s >= LO_SIZE]]) - HI_BASE
                lo_lists[c][b][p] = lo
                hi_lists[c][b][p] = hi
                Ka[b] = max(Ka[b], len(lo))
                Kb[b] = max(Kb[b], len(hi))

    # chunks of consecutive blocks with uniform KA/KB per chunk
    chunks = []
    cur = []
    for b in range(BPC):
        trial = cur + [b]
        ka = int(Ka[trial].max())
        kb = int(Kb[trial].max())
        if cur and len(trial) * (ka + kb) > S_MAX:
            chunks.append((cur, int(Ka[cur].max()), int(Kb[cur].max())))
            cur = [b]
        else:
            cur = trial
    if cur:
        chunks.append((cur, int(Ka[cur].max()), int(Kb[cur].max())))
    pr.chunks = [(list(blks), ka, kb) for blks, ka, kb in chunks]
    pr.n_slots = sum(len(blks) * (ka + kb) for blks, ka, kb in pr.chunks)

    fake_pos = np.where(node_of_pos < 0)[0]
    pad_lo = fake_pos[fake_pos < LO_SIZE]
    pad_hi = fake_pos[fake_pos >= HI_BASE] - HI_BASE
    assert len(pad_lo) and len(pad_hi)

    # index streams. T format (layers 1-2, transpose-mode gather): per chunk,
    # lo cols ordered (block, partition, k), then hi cols. N format (layer 3):
    # lo slots ordered (block, k) x 128 partitions, then hi.
    n_idx = pr.n_slots * 128
    idxT = np.empty((NCORES, n_idx), dtype=np.int64)
    idxN = np.empty((NCORES, n_idx), dtype=np.int64)
    spans = []   # per chunk: (lo0, n_lo, hi0, n_hi) in idx units
    i0 = 0
    for blks, ka, kb in pr.chunks:
        nb = len(blks)
        spans.append((i0, nb * 128 * ka, i0 + nb * 128 * ka, nb * 128 * kb))
        i0 += nb * 128 * (ka + kb)
    pr.call_spans = spans

    for c in range(NCORES):
        padk = 0
        i = 0
        for blks, ka, kb in pr.chunks:
            for lists, K, pads in ((lo_lists[c], ka, pad_lo),
                                   (hi_lists[c], kb, pad_hi)):
                base = i
                nb = len(blks)
                for bi, b in enumerate(blks):
                    for p in range(128):
                        lst = lists[b][p]
                        for k in range(K):
                            v = lst[k] if k < len(lst) else pads[padk % len(pads)]
                            if k >= len(lst):
                                padk += 1
                            idxT[c, base + (bi * 128 + p) * K + k] = v
                            idxN[c, base + (bi * K + k) * 128 + p] = v
                i += nb * 128 * K
        assert i == n_idx

    def pack(streams):
        ncols = n_idx // 16
        out = np.zeros((NCORES, 128, ncols), dtype=np.int16)
        ii = np.arange(n_idx)
        for c in range(NCORES):
            grp = np.zeros((16, ncols), dtype=np.int16)
            grp[ii % 16, ii // 16] = streams[c].astype(np.int16)
            for g in range(8):
                out[c, g * 16:(g + 1) * 16, :] = grp
        return out

    pr.idxT_packed = pack(idxT)
    pr.idxN_packed = pack(idxN)
    pr.ncols = n_idx // 16
    pr.idxT = idxT
    pr.idxN = idxN

    dinv_pos = np.zeros(NTAB, dtype=np.float32)
    real = node_of_pos >= 0
    dinv_pos[real] = dinv[node_of_pos[real]]
    pr.dinv_col = np.zeros((NCORES, 128, BPC), dtype=np.float32)
    pr.dinv_mat = np.zeros((NCORES, 128, NLOC), dtype=BF16)
    for c in range(NCORES):
        seg = dinv_pos[c * NLOC:(c + 1) * NLOC]
        pr.dinv_col[c] = seg.reshape(BPC, 128).T
        pr.dinv_mat[c] = np.broadcast_to(seg.astype(BF16), (128, NLOC))

    pr.dinv = dinv
    pr.node_of_pos = node_of_pos
    pr.tpos = tpos
    pr.xs = build_xs(pr, x)
    pr.xs_sh = [np.ascontiguousarray(pr.xs[c * NLOC:(c + 1) * NLOC]).astype(BF16)
                for c in range(NCORES)]
    return pr


def build_xs(pr: Prep, x: np.ndarray) -> np.ndarray:
    xs = np.zeros((NTAB, D), dtype=np.float32)
    xs[pr.tpos] = x * pr.dinv[:, None]
    return xs


# ---------------------------------------------------------------------------
# numpy emulator (validates prep/packing + the new layer algebra)
# ---------------------------------------------------------------------------

def emulate(pr: Prep, W0, b0, W1, b1, W2, b2) -> np.ndarray:
    tab = pr.xs.copy()
    out_full = np.zeros((NTAB, D), np.float32)
    for layer in range(3):
        new_tab = np.zeros((NTAB, D), np.float32)
        for c in range(NCORES):
            dv = pr.dinv_col[c].T.reshape(NLOC)     # dinv by table position
            for (blks, ka, kb), (lo0, nlo, hi0, nhi) in zip(pr.chunks, pr.call_spans):
                nb = len(blks)
                ilo = pr.idxT[c, lo0:lo0 + nlo].reshape(nb * 128, ka)
                ihi = pr.idxT[c, hi0:hi0 + nhi].reshape(nb * 128, kb)
                acc = (tab[:LO_SIZE][ilo].sum(axis=1, dtype=np.float32)
                       + tab[HI_BASE:][ihi].sum(axis=1, dtype=np.float32))
                for bi, b in enumerate(blks):
                    a = acc[bi * 128:(bi + 1) * 128]        # [128 dst, D]
                    d = dv[b * 128:(b + 1) * 128][:, None]
                    if layer == 0:
                        h = np.maximum((a @ W0) * d + b0[None, :], 0.0)
                        new_tab[c * NLOC + b * 128:c * NLOC + (b + 1) * 128] = (h @ W1) * d
                    elif layer == 1:
                        h = np.maximum(a * d + b1[None, :], 0.0)
                        new_tab[c * NLOC + b * 128:c * NLOC + (b + 1) * 128] = (h @ W2) * d
                    else:
                        h = np.maximum(a * d + b2[None, :], 0.0)
                        out_full[c * NLOC + b * 128:c * NLOC + (b + 1) * 128] = h
        tab = new_tab

    out = np.zeros((N, D), np.float32)
    pos = np.where(pr.node_of_pos >= 0)[0]
    out[pr.node_of_pos[pos]] = out_full[pos]
    return out


# ---------------------------------------------------------------------------
# bass kernel
# ---------------------------------------------------------------------------

def build_nc(pr: Prep, repeats: int = 1):
    import concourse.bacc as bacc
    import concourse.mybir as mybir
    import concourse.tile as tile

    f32 = mybir.dt.float32
    bf16 = mybir.dt.bfloat16
    nc = bacc.Bacc("TRN2", target_bir_lowering=False, debug=False,
                   num_devices=NCORES)

    xs_in = nc.dram_tensor("xs", [NLOC, D], bf16, kind="ExternalInput")
    idxT_in = nc.dram_tensor("idxT", [128, pr.ncols], mybir.dt.int16, kind="ExternalInput")
    idxN_in = nc.dram_tensor("idxN", [128, pr.ncols], mybir.dt.int16, kind="ExternalInput")
    dinv_col_in = nc.dram_tensor("dinv_col", [128, BPC], f32, kind="ExternalInput")
    dinv_mat_in = nc.dram_tensor("dinv_mat", [128, NLOC], bf16, kind="ExternalInput")
    W_in = [nc.dram_tensor(f"W{i}", [D, D], bf16, kind="ExternalInput") for i in range(3)]
    W032_in = nc.dram_tensor("W032", [D, D], f32, kind="ExternalInput")
    bcol_in = [nc.dram_tensor(f"bc{i}", [D, 1], f32, kind="ExternalInput") for i in range(3)]
    bmat_in = nc.dram_tensor("bmat2", [128, D], bf16, kind="ExternalInput")
    out = nc.dram_tensor("out", [NLOC, D], bf16, kind="ExternalOutput")

    xs_stage = nc.dram_tensor("xs_stage", [NLOC, D], bf16)
    bounce = [nc.dram_tensor(f"bounce{l}", [NLOC, D], bf16) for l in (2, 3)]
    tab_full = [nc.dram_tensor(f"tab{l}", [NTAB, D], bf16, addr_space="Shared")
                for l in (1, 2, 3)]

    with tile.TileContext(nc) as tc:
        with (
            tc.tile_pool(name="const", bufs=1) as cpool,
            tc.tile_pool(name="gpool", bufs=2) as gpool,
            tc.tile_pool(name="spool", bufs=1) as spool,
            tc.tile_pool(name="psum", bufs=2, space="PSUM") as ppool,
            tc.tile_pool(name="psum2", bufs=2, space="PSUM") as ppool2,
        ):
            idxT_sb = cpool.tile([128, pr.ncols], mybir.dt.int16, tag="ixT")
            nc.sync.dma_start(idxT_sb[:], idxT_in[:])
            idxN_sb = cpool.tile([128, pr.ncols], mybir.dt.int16, tag="ixN")
            nc.sync.dma_start(idxN_sb[:], idxN_in[:])
            dinv_col = cpool.tile([128, BPC], f32, tag="dc")
            nc.sync.dma_start(dinv_col[:], dinv_col_in[:])
            dinv_mat = cpool.tile([128, NLOC], bf16, tag="dm")
            nc.sync.dma_start(dinv_mat[:], dinv_mat_in[:])
            bmat2 = cpool.tile([128, D], bf16, tag="bm")
            nc.sync.dma_start(bmat2[:], bmat_in[:])
            W0_32 = cpool.tile([D, D], f32, tag="w032")
            nc.sync.dma_start(W0_32[:], W032_in[:])
            W_sb = []
            bcol_sb = []
            for i in range(3):
                w = cpool.tile([D, D], bf16, tag=f"w{i}")
                nc.sync.dma_start(w[:], W_in[i][:])
                W_sb.append(w)
                b = cpool.tile([D, 1], f32, tag=f"bb{i}")
                nc.sync.dma_start(b[:], bcol_in[i][:])
                bcol_sb.append(b)

            nc.sync.dma_start(xs_stage[:], xs_in[:])
            bypass = mybir.AluOpType.bypass
            add = mybir.AluOpType.add
            mult = mybir.AluOpType.mult
            amax = mybir.AluOpType.max

            for rep in range(repeats):
              nc.gpsimd.collective_compute(
                  "AllGather", bypass,
                  replica_groups=[list(range(NCORES))],
                  ins=[xs_stage[:]], outs=[tab_full[0][:]],
              )
              for layer in range(3):
                  tab = tab_full[layer]
                  tmode = layer < 2
                  idx_sb = idxT_sb if tmode else idxN_sb
                  for (blks, ka, kb), (lo0, nlo, hi0, nhi) in zip(pr.chunks, pr.call_spans):
                      nb = len(blks)
                      nd = nb * 128
                      ncol = nlo + nhi
                      if tmode:
                          GT = gpool.tile([128, ncol], bf16, tag="GT")
                          nc.gpsimd.dma_gather(
                              GT[:, 0:nlo].unsqueeze(1), tab[0:LO_SIZE, :],
                              idx_sb[:, lo0 // 16:(lo0 + nlo) // 16],
                              nlo, nlo, D, transpose=True, single_packet=False,
                          )
                          nc.gpsimd.dma_gather(
                              GT[:, nlo:ncol].unsqueeze(1), tab[HI_BASE:NTAB, :],
                              idx_sb[:, hi0 // 16:(hi0 + nhi) // 16],
                              nhi, nhi, D, transpose=True, single_packet=False,
                          )
                          accL = spool.tile([128, nd], f32, tag="accL")
                          nc.vector.tensor_reduce(
                              accL[:], GT[:, 0:nlo].rearrange("p (c k) -> p c k", k=ka),
                              mybir.AxisListType.X, add)
                          accH = spool.tile([128, nd], f32, tag="accH")
                          nc.vector.tensor_reduce(
                              accH[:], GT[:, nlo:ncol].rearrange("p (c k) -> p c k", k=kb),
                              mybir.AxisListType.X, add)
                          acc = spool.tile([128, nd], f32, tag="acc")
                          nc.vector.scalar_tensor_tensor(
                              acc[:], accL[:], 1.0, accH[:], bypass, add)
                          d0 = blks[0] * 128
                          hT = spool.tile([128, nd], bf16, tag="hT")
                          t = spool.tile([128, nd], f32, tag="t")
                          if layer == 0:
                              for s0 in range(0, nd, 512):
                                  w = min(512, nd - s0)
                                  hw = ppool.tile([128, 512], f32, tag="hw")
                                  nc.tensor.matmul(hw[:, 0:w], W0_32[:],
                                                   acc[:, s0:s0 + w],
                                                   start=True, stop=True)
                                  nc.vector.scalar_tensor_tensor(
                                      t[:, s0:s0 + w], hw[:, 0:w], 1.0,
                                      dinv_mat[:, d0 + s0:d0 + s0 + w],
                                      bypass, mult)
                          else:
                              nc.vector.scalar_tensor_tensor(
                                  t[:], acc[:], 1.0,
                                  dinv_mat[:, d0:d0 + nd], bypass, mult)
                          nc.vector.tensor_scalar(
                              hT[:], t[:], bcol_sb[layer][:], 0.0, add, amax)
                          # table rebuild: per dst block, tab_row = (h @ Wn) * dinv
                          Wn = W_sb[1] if layer == 0 else W_sb[2]
                          dst_dram = bounce[layer]
                          for g0 in range(0, nb, 4):
                              gn = min(4, nb - g0)
                              tp = ppool2.tile([128, 4 * 128], f32, tag="tp")
                              for gi in range(gn):
                                  nc.tensor.matmul(
                                      tp[:, gi * 128:(gi + 1) * 128],
                                      hT[:, (g0 + gi) * 128:(g0 + gi + 1) * 128],
                                      Wn[:], start=True, stop=True)
                              tabs = spool.tile([128, 4, 128], bf16, tag="tabs")
                              bsel = dinv_col[:, blks[0] + g0:blks[0] + g0 + gn]
                              nc.vector.scalar_tensor_tensor(
                                  tabs[:, 0:gn, :],
                                  tp[:, 0:gn * 128].rearrange("p (c f) -> p c f", f=128),
                                  1.0,
                                  bsel.unsqueeze(2).broadcast_to([128, gn, 128]),
                                  bypass, mult)
                              r0 = (blks[0] + g0) * 128
                              nc.sync.dma_start(
                                  dst_dram[r0:r0 + gn * 128, :].rearrange(
                                      "(c p) f -> p c f", c=gn),
                                  tabs[:, 0:gn, :])
                      else:
                          Gflat = gpool.tile([128, ncol], bf16, tag="GT")
                          GN = Gflat[:].rearrange("p (s f) -> p s f", f=D)
                          slo = nlo // 128
                          shi = nhi // 128
                          nc.gpsimd.dma_gather(
                              GN[:, 0:slo, :], tab[0:LO_SIZE, :],
                              idx_sb[:, lo0 // 16:(lo0 + nlo) // 16],
                              nlo, nlo, D, single_packet=False,
                          )
                          nc.gpsimd.dma_gather(
                              GN[:, slo:slo + shi, :], tab[HI_BASE:NTAB, :],
                              idx_sb[:, hi0 // 16:(hi0 + nhi) // 16],
                              nhi, nhi, D, single_packet=False,
                          )
                          accL = spool.tile([128, nb, D], f32, tag="accL")
                          nc.vector.tensor_reduce(
                              accL[:],
                              GN[:, 0:slo, :].rearrange("p (c k) f -> p c f k", k=ka),
                              mybir.AxisListType.X, add)
                          accH = spool.tile([128, nb, D], f32, tag="accH")
                          nc.vector.tensor_reduce(
                              accH[:],
                              GN[:, slo:slo + shi, :].rearrange("p (c k) f -> p c f k", k=kb),
                              mybir.AxisListType.X, add)
                          bsel = dinv_col[:, blks[0]:blks[0] + nb]
                          t1 = spool.tile([128, nb, D], f32, tag="acc")
                          nc.vector.scalar_tensor_tensor(
                              t1[:], accL[:], 1.0, accH[:], bypass, add)
                          t2 = spool.tile([128, nb, D], f32, tag="t")
                          nc.vector.scalar_tensor_tensor(
                              t2[:], t1[:], 1.0,
                              bsel.unsqueeze(2).broadcast_to([128, nb, 128]),
                              bypass, mult)
                          t3 = spool.tile([128, nb, D], f32, tag="accs")
                          nc.vector.scalar_tensor_tensor(
                              t3[:], t2[:], 1.0,
                              bmat2[:].unsqueeze(1).broadcast_to([128, nb, 128]),
                              bypass, add)
                          h2 = spool.tile([128, nb, D], bf16, tag="hT")
                          nc.vector.tensor_scalar(
                              h2[:], t3[:], 0.0, None, amax)
                          r0 = blks[0] * 128
                          nc.sync.dma_start(
                              out[r0:r0 + nb * 128, :].rearrange(
                                  "(c p) f -> p c f", c=nb),
                              h2[:])
                  if layer < 2:
                      nc.gpsimd.collective_compute(
                          "AllGather", bypass,
                          replica_groups=[list(range(NCORES))],
                          ins=[bounce[layer][:]], outs=[tab_full[layer + 1][:]],
                      )
    nc.compile()
    return nc


_CACHE = {}


def kernel(x, edge_index, W0, b0, W1, b1, W2, b2):
    from concourse.bass_utils import run_bass_kernel_spmd

    x = np.asarray(x, dtype=np.float32)
    edge_index = np.asarray(edge_index)
    ekey = hash(edge_index.tobytes())
    if _CACHE.get("ekey") == ekey:
        pr = _CACHE["pr"]
        if _CACHE.get("xkey") != hash(x.tobytes()):
            xs = build_xs(pr, x)
            pr.xs_sh = [np.ascontiguousarray(
                xs[c * NLOC:(c + 1) * NLOC]).astype(BF16)
                for c in range(NCORES)]
            _CACHE["xkey"] = hash(x.tobytes())
    else:
        _CACHE.pop("pr", None)
        for k in [k for k in _CACHE if isinstance(k, tuple) and k[0] == "nc"]:
            _CACHE.pop(k)
        pr = _CACHE["pr"] = preprocess(x, edge_index)
        _CACHE["ekey"] = ekey
        _CACHE["xkey"] = hash(x.tobytes())

    repeats = int(os.environ.get("GCN_REPEATS", "1"))
    key = ("nc", repeats)
    if key not in _CACHE:
        _CACHE[key] = build_nc(pr, repeats)
    nc = _CACHE[key]

    Ws = [np.asarray(w, np.float32).astype(BF16) for w in (W0, W1, W2)]
    bs = [np.asarray(b, np.float32) for b in (b0, b1, b2)]
    in_maps = []
    for c in range(NCORES):
        m = {
            "xs": pr.xs_sh[c],
            "idxT": pr.idxT_packed[c],
            "idxN": pr.idxN_packed[c],
            "dinv_col": pr.dinv_col[c],
            "dinv_mat": pr.dinv_mat[c],
            "bmat2": np.broadcast_to(bs[2].astype(BF16), (128, D)).copy(),
        }
        m["W032"] = np.asarray(W0, np.float32)
        for i in range(3):
            m[f"W{i}"] = Ws[i]
            m[f"bc{i}"] = np.ascontiguousarray(bs[i].reshape(D, 1))
        in_maps.append(m)

    res = run_bass_kernel_spmd(nc, in_maps, core_ids=list(range(NCORES)))
    kernel.last_results = res

    out = np.zeros((N, D), np.float32)
    for c in range(NCORES):
        pos = np.where(pr.node_of_pos[c * NLOC:(c + 1) * NLOC] >= 0)[0]
        out[pr.node_of_pos[c * NLOC + pos]] = (
            np.asarray(res.results[c]["out"][pos]).astype(np.float32))
    return out
